# revision 1
# baseline (speedup 1.0000x reference)
"""Multi-head attention Trainium2 Bass kernel.

Problem: B=2, S=2048, D=1024, H=16 heads, DH=64, causal (or arbitrary) mask.
Sharding: 8 cores = data-parallel over B (2) x tensor-parallel over head
groups (4 groups of 4 heads). Each core computes QKV projections for its
head group, attention for its 4 heads, and a partial output projection
(attended @ Wo-shard). Host sums the 4 partials per batch and adds bo.

Core-local design ("transposed scores" formulation, all matmuls float32r):
  xT   [1024, S]    PE transposes of x tiles
  qT,kT [256, S]    W.T @ xT
  v'   [S, 4, 65]   v in natural layout + a ones column per head (makes the
                    PV matmul emit the softmax denominator as row 64)
  sT   [kv, q]      scores transposed = kT-chunk.T @ qT; K=64 matmuls are
                    row-tiled (two heads run on the two 64-row PE tiles)
  expT exp(s/8)     f32r; causality: kv-blocks > q skipped, ranges
                    restricted, upper-tri 0/1 template on diagonal blocks
  attT [65, q]      v'.T @ expT accumulated over kv chunks, K split in two
                    64-row halves into separate PSUM banks, summed on DVE;
                    row 64 = softmax denom; normalize via reciprocal +
                    partition_broadcast + multiply -> attT [128, S] f32r
  out  [S, 1024]    attT.T @ Wo-shard (partial; host reduces over cores)
"""
import numpy as np

B, S, D = 2, 2048, 1024
H, DH = 16, 64
NCORES = 8
HPC = 4              # heads per core
DIMS = HPC * DH      # 256 projection dims per core
NSB = S // 512       # 4 q/s blocks of 512
NST = S // 128       # 16 s tiles of 128
NDC = D // 128       # 8 contraction chunks

_PROG_CACHE = {}


def build_program(mode="causal", has_bias=False, reps=1,
                  phases=("tp", "qkv", "attn", "proj")):
    import concourse.bacc as bacc
    import concourse.mybir as mybir
    import concourse.tile as tile

    DT = mybir.dt.float32
    FR = mybir.dt.float32r
    Act = mybir.ActivationFunctionType
    MUL = mybir.AluOpType.mult
    ADD = mybir.AluOpType.add

    nc = bacc.Bacc("TRN2", target_bir_lowering=False, debug=False,
                   num_devices=NCORES)

    d_in = D + 1 if has_bias else D
    xin = nc.dram_tensor("xin", [S, D], FR, kind="ExternalInput")
    wq = nc.dram_tensor("wq", [d_in, DIMS], FR, kind="ExternalInput")
    wk = nc.dram_tensor("wk", [d_in, DIMS], FR, kind="ExternalInput")
    wv = nc.dram_tensor("wv", [d_in, DIMS], FR, kind="ExternalInput")
    wo = nc.dram_tensor("wo", [DIMS, D], FR, kind="ExternalInput")
    # consts: [:,0:128] identity | [:,128:256] upper-tri ones (diag incl.)
    # | [:,256:260] ones | [:,260:772] = [384 zero cols | 128 tri cols]
    # (causal kill-template for partial kv chunks)
    # | has_bias: [:,772:1284] ones (row 0 = x^T ones row)
    ncst = 1284 if has_bias else 772
    cin = nc.dram_tensor("cin", [128, ncst], FR, kind="ExternalInput")
    if mode == "general":
        mtin = nc.dram_tensor("maskt", [S, S], FR, kind="ExternalInput")
    outp = nc.dram_tensor("outp", [S, D], DT, kind="ExternalOutput")

    with tile.TileContext(nc) as tc:
        with (
            tc.tile_pool(name="pw", bufs=1) as pw,
            tc.tile_pool(name="px", bufs=5) as px,
            tc.tile_pool(name="pxt", bufs=10) as pxt,
            tc.tile_pool(name="pqk", bufs=1) as pqk,
            tc.tile_pool(name="pv", bufs=1) as pv,
            tc.tile_pool(name="pexp", bufs=6) as pexp,
            tc.tile_pool(name="pmask", bufs=4) as pmask,
            tc.tile_pool(name="patt", bufs=1) as patt,
            tc.tile_pool(name="pout", bufs=3) as pout,
            tc.tile_pool(name="pmisc", bufs=4) as pmisc,
            tc.tile_pool(name="psA", bufs=2, space="PSUM") as psA,   # 2-bank slots
            tc.tile_pool(name="psB", bufs=4, space="PSUM") as psB,   # 1-bank slots, shared

        ):
            def body():
                for _ in range(2):
                    pini = psA.tile([128, 2, 512], DT, tag="sc")
                    nc.vector.memset(pini[:], 0.0)
                # ---- weights + consts ----
                wqt = pw.tile([128, NDC, DIMS], FR, tag="wqt")
                wkt = pw.tile([128, NDC, DIMS], FR, tag="wkt")
                wvt = pw.tile([128, NDC, DIMS], FR, tag="wvt")
                wot = pw.tile([128, 2, D], FR, tag="wot")
                cst = pw.tile([128, ncst], FR, tag="cst")
                for d in range(NDC):
                    nc.sync.dma_start(wqt[:, d, :], wq[d * 128:(d + 1) * 128, :])
                    nc.sync.dma_start(wkt[:, d, :], wk[d * 128:(d + 1) * 128, :])
                    nc.sync.dma_start(wvt[:, d, :], wv[d * 128:(d + 1) * 128, :])
                for t in range(2):
                    nc.sync.dma_start(wot[:, t, :], wo[t * 128:t * 128 + 128, :])
                nc.sync.dma_start(cst[:], cin[:])
                ident = cst[:, 0:128]
                tri = cst[:, 128:256]
                if has_bias:
                    wqb = pw.tile([1, DIMS], FR, tag="wqb")
                    wkb = pw.tile([1, DIMS], FR, tag="wkb")
                    wvb = pw.tile([1, DIMS], FR, tag="wvb")
                    nc.sync.dma_start(wqb[:], wq[D:D + 1, :])
                    nc.sync.dma_start(wkb[:], wk[D:D + 1, :])
                    nc.sync.dma_start(wvb[:], wv[D:D + 1, :])
                    bias_lhs = {"q": wqb, "k": wkb}

                if "qkv" in phases:
                    qTs = [pqk.tile([128, S], FR, tag=f"qT{t}", name=f"qTs{t}") for t in range(2)]
                    kTs = [pqk.tile([128, S], FR, tag=f"kT{t}", name=f"kTs{t}") for t in range(2)]
                    v_all = pv.tile([128, NST, HPC, 65], FR, tag="v")
                if "attn" in phases:
                    attTs = [patt.tile([128, S], FR, tag=f"aT{t}", name=f"attTs{t}") for t in range(2)]

                for sb in range(NSB):
                    # ---- x tiles + xT slices for this s-block ----
                    xtiles = []
                    for stl in range(4):
                        st = sb * 4 + stl
                        xt_ = px.tile([128, D], FR, tag="x")
                        nc.sync.dma_start(xt_[:], xin[st * 128:(st + 1) * 128, :])
                        xtiles.append(xt_)
                    if "tp" not in phases:
                        junk = pout.tile([128, D], DT, tag="out")
                        for stl in range(4):
                            nc.vector.tensor_copy(junk[:, stl * 16:stl * 16 + 16],
                                                  xtiles[stl][:, 0:16].bitcast(DT))
                        nc.sync.dma_start(outp[sb * 128:(sb + 1) * 128, :], junk[:])
                        continue
                    pxt_tiles = []
                    for d in range(NDC):
                        tp = psB.tile([128, 512], FR, tag="b1")
                        for stl in range(4):
                            nc.tensor.transpose(
                                tp[:, stl * 128:(stl + 1) * 128],
                                xtiles[stl][:, d * 128:(d + 1) * 128], ident)
                        xts = pxt.tile([128, 512], FR, tag="xts")
                        nc.vector.tensor_copy(xts[:], tp[:])
                        pxt_tiles.append(xts)

                    if "qkv" not in phases:
                        junk = pout.tile([128, D], DT, tag="out")
                        for d in range(NDC):
                            nc.vector.tensor_copy(junk[:, d * 16:d * 16 + 16],
                                                  pxt_tiles[d][:, 0:16].bitcast(DT))
                        nc.sync.dma_start(outp[sb * 128:(sb + 1) * 128, :], junk[:])
                        continue
                    # ---- qT / kT projections ----
                    for wname, w3, dst in (("q", wqt, qTs), ("k", wkt, kTs)):
                        for t in range(2):
                            ps = psB.tile([128, 512], DT, tag="b1")
                            for d in range(NDC):
                                nc.tensor.matmul(
                                    ps[:], w3[:, d, t * 128:(t + 1) * 128],
                                    pxt_tiles[d][:],
                                    start=(d == 0),
                                    stop=(d == NDC - 1 and not has_bias))
                            if has_bias:
                                nc.tensor.matmul(
                                    ps[:],
                                    bias_lhs[wname][:, t * 128:(t + 1) * 128],
                                    cst[0:1, 772:1284],
                                    start=False, stop=True)
                            nc.vector.tensor_copy(
                                dst[t][:, sb * 512:(sb + 1) * 512], ps[:])

                    # ---- v (natural layout + ones column) ----
                    for stl in range(4):
                        st = sb * 4 + stl
                        ps = psB.tile([128, DIMS], DT, tag="b1")
                        for d in range(NDC):
                            nc.tensor.matmul(
                                ps[:], pxt_tiles[d][:, stl * 128:(stl + 1) * 128],
                                wvt[:, d, :],
                                start=(d == 0),
                                stop=(d == NDC - 1 and not has_bias))
                        if has_bias:
                            nc.tensor.matmul(
                                ps[:], cst[0:1, 772:900], wvb[:],
                                start=False, stop=True)
                        nc.vector.tensor_copy(
                            v_all[:, st, :, 0:64],
                            ps[:].rearrange("p (h e) -> p h e", h=HPC))
                        nc.gpsimd.tensor_copy(v_all[:, st, :, 64], cst[:, 256:260])

                    if "attn" not in phases:
                        junk = pout.tile([128, D], DT, tag="out")
                        nc.vector.tensor_copy(junk[:, 0:512],
                                              qTs[0][:, sb * 512:(sb + 1) * 512].bitcast(DT))
                        nc.vector.tensor_copy(junk[:, 512:1024],
                                              kTs[0][:, sb * 512:(sb + 1) * 512].bitcast(DT))
                        nc.vector.tensor_tensor(
                            junk[:, 0:512],
                            qTs[1][:, sb * 512:(sb + 1) * 512].bitcast(DT),
                            kTs[1][:, sb * 512:(sb + 1) * 512].bitcast(DT), MUL)
                        nc.vector.tensor_copy(
                            junk[:, 0:260],
                            v_all[:, sb, :, :].rearrange("p a b -> p (a b)").bitcast(DT))
                        nc.sync.dma_start(outp[sb * 128:(sb + 1) * 128, :], junk[:])
                        continue
                    # ---- attention for q-block qb = sb ----
                    qb = sb
                    nkv = 4 * qb + 4 if mode == "causal" else NST
                    ncg = nkv // 2
                    q0 = qb * 512
                    for pair in range(2):
                        accs = [psB.tile([65, 512], DT, tag="b1",
                                         name=f"acc{i}")
                                for i in range(2)]
                        # software pipeline: PV lags scores/exp by LAG groups
                        LAG = 2
                        exp_q = {}
                        for u in range(ncg + LAG):
                            if u < ncg:
                                cg = u
                                if mode == "general":
                                    mt = pmask.tile([128, 2, 512], FR, tag="mt")
                                    for j in range(2):
                                        c = 2 * cg + j
                                        nc.sync.dma_start(
                                            mt[:, j, :],
                                            mtin[c * 128:(c + 1) * 128, q0:q0 + 512])
                                for h2 in range(2):
                                    lo, hi = h2 * 64, (h2 + 1) * 64
                                    scps = psA.tile([128, 2, 512], DT, tag="sc")
                                    for j in range(2):
                                        c = 2 * cg + j
                                        off = (max(0, c * 128 - q0)
                                               if mode == "causal" else 0)
                                        nc.tensor.matmul(
                                            scps[:, j, off:512],
                                            kTs[pair][lo:hi, c * 128:(c + 1) * 128],
                                            qTs[pair][lo:hi, q0 + off:q0 + 512],
                                            start=True, stop=True)
                                    expt = pexp.tile([128, 2, 512], FR, tag="exp")
                                    nc.scalar.activation(expt[:], scps[:], Act.Exp,
                                                         scale=float(1.0 / np.sqrt(DH)))
                                    for j in range(2):
                                        c = 2 * cg + j
                                        if mode == "causal" and c * 128 >= q0:
                                            off = c * 128 - q0
                                            nc.gpsimd.tensor_tensor(
                                                expt[:, j, 0:off + 128],
                                                expt[:, j, 0:off + 128],
                                                cst[:, 644 - off:772], MUL)
                                        elif mode == "general":
                                            nc.gpsimd.tensor_tensor(
                                                expt[:, j, :], expt[:, j, :],
                                                mt[:, j, :], MUL)
                                    exp_q[(cg, h2)] = expt
                            if u >= LAG:
                                cg = u - LAG
                                for h2 in range(2):
                                    expt = exp_q.pop((cg, h2))
                                    h = pair * 2 + h2
                                    for j in range(2):
                                        c = 2 * cg + j
                                        nc.tensor.matmul(
                                            accs[h2][:],
                                            v_all[:, c, h, :],
                                            expt[:, j, :],
                                            start=(c == 0), stop=(c == nkv - 1))
                        for h2 in range(2):
                            recip = pmisc.tile([1, 512], DT, tag="recip")
                            nc.vector.reciprocal(recip[:], accs[h2][64:65, :])
                            recipb = pmisc.tile([64, 512], DT, tag="recipb")
                            nc.gpsimd.partition_broadcast(recipb[:], recip[:])
                            nc.vector.tensor_tensor(
                                attTs[pair][h2 * 64:(h2 + 1) * 64, q0:q0 + 512],
                                accs[h2][0:64, :], recipb[:], MUL)

                if "proj" not in phases:
                    if "attn" in phases:
                        for t in range(2):
                            for rr in range(2):
                                nc.sync.dma_start(
                                    outp[(t * 2 + rr) * 128:(t * 2 + rr + 1) * 128, :],
                                    attTs[t][:, rr * 1024:(rr + 1) * 1024].bitcast(DT))
                    return
                # ---- output projection (partial; host reduces) ----
                for st in range(NST):
                    ps = psA.tile([128, 2, 512], DT, tag="sc")
                    for half in range(2):
                        for t in range(2):
                            nc.tensor.matmul(
                                ps[:, half, :],
                                attTs[t][:, st * 128:(st + 1) * 128],
                                wot[:, t, half * 512:(half + 1) * 512],
                                start=(t == 0), stop=(t == 1))
                    ot = pout.tile([128, D], DT, tag="out")
                    nc.vector.tensor_copy(ot[:], ps[:].rearrange("p a b -> p (a b)"))
                    nc.sync.dma_start(outp[st * 128:(st + 1) * 128, :], ot[:])

            if reps == 1:
                body()
            else:
                with tc.For_i(0, reps, 1):
                    body()

    nc.compile()
    return nc


def _consts_array(has_bias):
    ncst = 1284 if has_bias else 772
    c = np.zeros((128, ncst), dtype=np.float32)
    c[:, 0:128] = np.eye(128, dtype=np.float32)
    c[:, 128:256] = np.triu(np.ones((128, 128), np.float32))
    c[:, 256:260] = 1.0
    c[:, 644:772] = np.triu(np.ones((128, 128), np.float32))
    if has_bias:
        c[:, 772:1284] = 1.0
    return c


def make_in_maps(x, mask, Wq, bq, Wk, bk, Wv, bv, Wo, bo):
    x = np.asarray(x, np.float32)
    m = np.asarray(mask)[0, 0]
    mb = (m != 0)
    if mb.all():
        mode = "none"
    elif np.array_equal(mb, np.tril(np.ones((S, S), bool))):
        mode = "causal"
    else:
        mode = "general"
    has_bias = bool(np.any(bq) or np.any(bk) or np.any(bv))

    Wq = np.asarray(Wq, np.float32)
    Wk = np.asarray(Wk, np.float32)
    Wv = np.asarray(Wv, np.float32)
    Wo = np.asarray(Wo, np.float32)
    if has_bias:
        Wq = np.concatenate([Wq, np.asarray(bq, np.float32)[None, :]], 0)
        Wk = np.concatenate([Wk, np.asarray(bk, np.float32)[None, :]], 0)
        Wv = np.concatenate([Wv, np.asarray(bv, np.float32)[None, :]], 0)
    consts = _consts_array(has_bias)
    maskt = (np.ascontiguousarray(mb.T.astype(np.float32))
             if mode == "general" else None)

    in_maps = []
    for c in range(NCORES):
        b, hg = divmod(c, HPC)
        cols = slice(hg * DIMS, (hg + 1) * DIMS)
        im = {
            "xin": np.ascontiguousarray(x[b]),
            "wq": np.ascontiguousarray(Wq[:, cols]),
            "wk": np.ascontiguousarray(Wk[:, cols]),
            "wv": np.ascontiguousarray(Wv[:, cols]),
            "wo": np.ascontiguousarray(Wo[hg * DIMS:(hg + 1) * DIMS, :]),
            "cin": consts,
        }
        if maskt is not None:
            im["maskt"] = maskt
        in_maps.append(im)
    return in_maps, mode, has_bias


def gather_output(results, bo):
    out = np.zeros((B, S, D), dtype=np.float32)
    for c in range(NCORES):
        out[c // HPC] += results[c]["outp"]
    out += np.asarray(bo, np.float32)[None, None, :]
    return out


def run(in_maps, mode, has_bias, reps=1, phases=("tp", "qkv", "attn", "proj")):
    from concourse.bass_utils import run_bass_kernel_spmd
    key = (mode, has_bias, reps, tuple(phases))
    if key not in _PROG_CACHE:
        _PROG_CACHE[key] = build_program(mode, has_bias, reps, phases)
    nc = _PROG_CACHE[key]
    return run_bass_kernel_spmd(nc, in_maps, core_ids=list(range(NCORES)))


def kernel(x, mask, Wq, bq, Wk, bk, Wv, bv, Wo, bo):
    in_maps, mode, has_bias = make_in_maps(x, mask, Wq, bq, Wk, bk, Wv, bv,
                                           Wo, bo)
    r = run(in_maps, mode, has_bias, reps=1)
    return gather_output(r.results, bo)



# revision 24
# speedup vs baseline: 315.7711x; 315.7711x over previous
"""Multi-head attention Trainium2 Bass kernel (v2, bf16).

Problem: B=2, S=2048, D=1024, H=16 heads, DH=64, causal (or arbitrary) mask.
Sharding: 8 cores = data-parallel over B (2) x tensor-parallel over head
groups (4 groups of 4 heads). Each core computes QKV projections for its
head group, attention for its 4 heads, and a partial output projection
(attended @ Wo-shard). Host sums the 4 partials per batch and adds bo.

v2 vs v1:
  - all matmul operands bf16 (f32 PSUM); exp output bf16; output bf16
    (host upcasts + reduces)
  - x is pre-transposed on the host -> no PE transposes, no DVE xT copies
  - program order interleaves qkv(sb+1) and proj(qb-1) with the ACT-bound
    attention(qb) so the PE always has filler work
  - PSUM budget: scores 2x[128,2,512] (4 banks) + accs 2x[65,512] (2) +
    qkv/proj 2x[128,512] (2) = 8 banks

Core-local design ("transposed scores" formulation):
  xT   [1024, S]    bf16, host-pretransposed, DMA'd per (d, sb) chunk
  qT,kT [256, S]    W.T @ xT  (2 tiles of [128, S] per, one per head-pair)
  v'   [S, 4, 65]   v natural + ones column per head (PV matmul emits the
                    softmax denominator as row 64)
  sT   [kv, q]      scores transposed = kT-chunk.T @ qT; K=64 matmuls
                    row-tiled (two heads on the two 64-row PE tile rows)
  expT exp(s/8)     bf16; causality: kv-blocks > q skipped, ranges
                    restricted, upper-tri 0/1 template on diagonal blocks
  attT [65, q]      v'.T @ expT accumulated over kv chunks; row 64 =
                    denom; normalize via reciprocal + partition_broadcast
                    + multiply -> attT [128, S] bf16
  out  [S, 1024]    attT.T @ Wo-shard bf16 (partial; host reduces)
"""
import numpy as np

B, S, D = 2, 2048, 1024
H, DH = 16, 64
NCORES = 8
HPC = 4              # heads per core
DIMS = HPC * DH      # 256 projection dims per core
NSB = S // 512       # 4 q/s blocks of 512
NST = S // 128       # 16 s tiles of 128
NDC = D // 128       # 8 contraction chunks

_PROG_CACHE = {}


def build_program(mode="causal", has_bias=False, reps=1, phases=None):
    import concourse.bacc as bacc
    import concourse.mybir as mybir
    import concourse.tile as tile

    DT = mybir.dt.float32
    BF = mybir.dt.bfloat16
    Act = mybir.ActivationFunctionType
    MUL = mybir.AluOpType.mult

    nc = bacc.Bacc("TRN2", target_bir_lowering=False, debug=False,
                   num_devices=NCORES)

    ndc = NDC + 1 if has_bias else NDC
    d_in = ndc * 128
    xt = nc.dram_tensor("xt", [d_in, S], BF, kind="ExternalInput")
    wq = nc.dram_tensor("wq", [d_in, DIMS], BF, kind="ExternalInput")
    wk = nc.dram_tensor("wk", [d_in, DIMS], BF, kind="ExternalInput")
    wv = nc.dram_tensor("wv", [d_in, DIMS], BF, kind="ExternalInput")
    wo = nc.dram_tensor("wo", [DIMS, D], BF, kind="ExternalInput")
    # consts: [:,0:4] ones | [:,4:388] zeros | [:,388:516] upper-tri ones
    # | [:,516:4612] ones (one-shot DMA into v_all's denominator columns)
    cin = nc.dram_tensor("cin", [128, 4612], BF, kind="ExternalInput")
    if mode == "general":
        mtin = nc.dram_tensor("maskt", [S, S], BF, kind="ExternalInput")
    outp = nc.dram_tensor("outp", [S, D], BF, kind="ExternalOutput")

    with tile.TileContext(nc) as tc:
        with (
            tc.tile_pool(name="pw", bufs=1) as pw,
            tc.tile_pool(name="pxa", bufs=1) as pxa,
            tc.tile_pool(name="pqk", bufs=1) as pqk,
            tc.tile_pool(name="pv", bufs=1) as pv,
            tc.tile_pool(name="patt", bufs=1) as patt,
            tc.tile_pool(name="pexp", bufs=8) as pexp,
            tc.tile_pool(name="pmask", bufs=4) as pmask,
            tc.tile_pool(name="pmisc", bufs=4) as pmisc,
            tc.tile_pool(name="pout", bufs=4) as pout,
            tc.tile_pool(name="psS", bufs=2, space="PSUM") as psS,
            tc.tile_pool(name="psA", bufs=2, space="PSUM") as psA,
            tc.tile_pool(name="psQ", bufs=2, space="PSUM") as psQ,
        ):
            def body():
                # zero the score-psum slots: exp reads full banks while the
                # causal matmuls write only [off:512), so first use would
                # otherwise read uninitialized PSUM (exp -> inf -> NaN)
                for _ in range(2):
                    pini = psS.tile([128, 2, 512], DT, tag="sc")
                    nc.vector.memset(pini[:], 0.0)
                # ---- weights + consts ----
                wqt = pw.tile([128, ndc, DIMS], BF, tag="wqt")
                wkt = pw.tile([128, ndc, DIMS], BF, tag="wkt")
                wvt = pw.tile([128, ndc, DIMS], BF, tag="wvt")
                wot = pw.tile([128, 2, D], BF, tag="wot")
                cst = pw.tile([128, 516], BF, tag="cst")
                # qk weights + x(0) first so the first matmuls start ASAP
                nc.sync.dma_start(wqt[:], wq.rearrange("(d p) j -> p d j", p=128))
                nc.sync.dma_start(wkt[:], wk.rearrange("(d p) j -> p d j", p=128))

                xall = pxa.tile([128, ndc, S], BF, tag="xall")
                qTs = [pqk.tile([128, S], BF, tag=f"qT{t}", name=f"qTs{t}")
                       for t in range(2)]
                kTs = [pqk.tile([128, S], BF, tag=f"kT{t}", name=f"kTs{t}")
                       for t in range(2)]
                # v' = [v | 64 ones cols]: the PV matmul then emits the
                # softmax denominator pre-replicated on partitions 64:128,
                # so no gpsimd partition_broadcast is needed.
                v_all = pv.tile([128, NST, HPC, 128], BF, tag="v")
                nc.sync.dma_start(
                    v_all[:, :, :, 64:128],
                    cin[:, 516:4612].rearrange("p (s h e) -> p s h e",
                                               s=NST, h=HPC))
                attTs = [patt.tile([128, S], BF, tag=f"aT{t}", name=f"attTs{t}")
                         for t in range(2)]

                xt_r = xt.rearrange("(d p) s -> p d s", p=128)

                def load_x(sb):
                    c0 = sb * 512
                    nc.sync.dma_start(xall[:, :, c0:c0 + 512],
                                      xt_r[:, :, c0:c0 + 512])

                def qkv(sb):
                    # two psum chains interleaved -> alternating-bank matmuls
                    c0 = sb * 512
                    for w3, dst in ((wqt, qTs), (wkt, kTs)):
                        pss = [psQ.tile([128, 512], DT, tag="mm",
                                        name=f"qk{t}") for t in range(2)]
                        for d in range(ndc):
                            for t in range(2):
                                nc.tensor.matmul(
                                    pss[t][:], w3[:, d, t * 128:(t + 1) * 128],
                                    xall[:, d, c0:c0 + 512],
                                    start=(d == 0), stop=(d == ndc - 1))
                        for t in range(2):
                            nc.vector.tensor_copy(dst[t][:, c0:c0 + 512],
                                                  pss[t][:])
                    for sp in range(2):
                        sts = (sb * 4 + sp * 2, sb * 4 + sp * 2 + 1)
                        pss = [psQ.tile([128, 256], DT, tag="mm",
                                        name=f"v{i}") for i in range(2)]
                        for d in range(ndc):
                            for i, st in enumerate(sts):
                                nc.tensor.matmul(
                                    pss[i][:],
                                    xall[:, d, st * 128:(st + 1) * 128],
                                    wvt[:, d, :],
                                    start=(d == 0), stop=(d == ndc - 1))
                        for i, st in enumerate(sts):
                            nc.vector.tensor_copy(
                                v_all[:, st, :, 0:64],
                                pss[i][:].rearrange("p (h e) -> p h e", h=HPC))

                def attention(qb, pair):
                    nkv = 4 * qb + 4 if mode == "causal" else NST
                    ncg = nkv // 2
                    q0 = qb * 512
                    accs = [psA.tile([128, 512], DT, tag="acc", name=f"acc{i}")
                            for i in range(2)]
                    LAG = 2
                    exp_q = {}
                    for u in range(ncg + LAG):
                        if u < ncg:
                            cg = u
                            if mode == "general":
                                mt = pmask.tile([128, 2, 512], BF, tag="mt")
                                for j in range(2):
                                    c = 2 * cg + j
                                    nc.sync.dma_start(
                                        mt[:, j, :],
                                        mtin[c * 128:(c + 1) * 128, q0:q0 + 512])
                            for h2 in range(2):
                                lo, hi = h2 * 64, (h2 + 1) * 64
                                scps = psS.tile([128, 2, 512], DT, tag="sc")
                                for j in range(2):
                                    c = 2 * cg + j
                                    off = (max(0, c * 128 - q0)
                                           if mode == "causal" else 0)
                                    nc.tensor.matmul(
                                        scps[:, j, off:512],
                                        kTs[pair][lo:hi, c * 128:(c + 1) * 128],
                                        qTs[pair][lo:hi, q0 + off:q0 + 512],
                                        start=True, stop=True)
                                expt = pexp.tile([128, 2, 512], BF, tag="exp")
                                nc.scalar.activation(expt[:], scps[:], Act.Exp,
                                                     scale=0.125)
                                # kill-multiply on DVE (bf16 2x mode); the
                                # gpsimd engine is far slower than modeled
                                eng = nc.vector
                                for j in range(2):
                                    c = 2 * cg + j
                                    if mode == "causal" and c * 128 >= q0:
                                        off = c * 128 - q0
                                        eng.tensor_tensor(
                                            expt[:, j, 0:off + 128],
                                            expt[:, j, 0:off + 128],
                                            cst[:, 388 - off:516], MUL)
                                    elif mode == "general":
                                        eng.tensor_tensor(
                                            expt[:, j, :], expt[:, j, :],
                                            mt[:, j, :], MUL)
                                exp_q[(cg, h2)] = expt
                        if u >= LAG:
                            cg = u - LAG
                            es = [exp_q.pop((cg, 0)), exp_q.pop((cg, 1))]
                            # j-outer so consecutive matmuls alternate the
                            # two accumulator banks
                            for j in range(2):
                                c = 2 * cg + j
                                for h2 in range(2):
                                    nc.tensor.matmul(
                                        accs[h2][:],
                                        v_all[:, c, pair * 2 + h2, :],
                                        es[h2][:, j, :],
                                        start=(c == 0), stop=(c == nkv - 1))
                    for h2 in range(2):
                        # accs rows 64:128 hold the denominator (replicated
                        # by the ones columns of v'); normalize on DVE only
                        if phases == "nonorm":
                            nc.vector.tensor_copy(
                                attTs[pair][h2 * 64:(h2 + 1) * 64,
                                            q0:q0 + 512],
                                accs[h2][0:64, :])
                            continue
                        if phases == "denom":
                            nc.vector.tensor_copy(
                                attTs[pair][h2 * 64:(h2 + 1) * 64,
                                            q0:q0 + 512],
                                accs[h2][64:128, :])
                            continue
                        recipb = pmisc.tile([64, 512], DT, tag="recipb")
                        nc.vector.reciprocal(recipb[:], accs[h2][64:128, :])
                        nc.vector.tensor_tensor(
                            attTs[pair][h2 * 64:(h2 + 1) * 64, q0:q0 + 512],
                            accs[h2][0:64, :], recipb[:], MUL)

                def proj(qb):
                    # t-outer so the attT stationary is loaded once per two
                    # matmuls (two psum chains, one per output half)
                    for stl in range(4):
                        st = qb * 4 + stl
                        ot = pout.tile([128, D], BF, tag="out")
                        pss = [psQ.tile([128, 512], DT, tag="mm",
                                        name=f"pj{half}") for half in range(2)]
                        for t in range(2):
                            for half in range(2):
                                nc.tensor.matmul(
                                    pss[half][:],
                                    attTs[t][:, st * 128:(st + 1) * 128],
                                    wot[:, t, half * 512:(half + 1) * 512],
                                    start=(t == 0), stop=(t == 1))
                        for half in range(2):
                            nc.vector.tensor_copy(
                                ot[:, half * 512:(half + 1) * 512],
                                pss[half][:])
                        nc.sync.dma_start(outp[st * 128:(st + 1) * 128, :],
                                          ot[:])

                load_x(0)
                nc.sync.dma_start(wvt[:], wv.rearrange("(d p) j -> p d j", p=128))
                nc.sync.dma_start(wot[:], wo.rearrange("(t p) j -> p t j", p=128))
                nc.sync.dma_start(cst[:], cin[:, 0:516])
                qkv(0)
                load_x(1); qkv(1)
                attention(0, 0); attention(0, 1)
                load_x(2); qkv(2)
                attention(1, 0); attention(1, 1)
                load_x(3); qkv(3)
                proj(0)
                attention(2, 0); attention(2, 1)
                proj(1)
                attention(3, 0)
                proj(2)
                attention(3, 1)
                proj(3)

            if reps == 1:
                body()
            else:
                with tc.For_i(0, reps, 1):
                    body()

    nc.compile()
    return nc


def _consts_array():
    c = np.zeros((128, 4612), dtype=np.float32)
    c[:, 0:4] = 1.0
    c[:, 388:516] = np.triu(np.ones((128, 128), np.float32))
    c[:, 516:4612] = 1.0
    return c


def _bf16(a):
    import ml_dtypes
    return np.ascontiguousarray(np.asarray(a, np.float32)).astype(
        ml_dtypes.bfloat16)


def make_in_maps(x, mask, Wq, bq, Wk, bk, Wv, bv, Wo, bo):
    x = np.asarray(x, np.float32)
    m = np.asarray(mask)[0, 0]
    mb = (m != 0)
    if mb.all():
        mode = "none"
    elif np.array_equal(mb, np.tril(np.ones((S, S), bool))):
        mode = "causal"
    else:
        mode = "general"
    has_bias = bool(np.any(bq) or np.any(bk) or np.any(bv))

    Wq = np.asarray(Wq, np.float32)
    Wk = np.asarray(Wk, np.float32)
    Wv = np.asarray(Wv, np.float32)
    Wo = np.asarray(Wo, np.float32)
    if has_bias:
        # contraction dim padded to 9*128: row D = bias (ones row in xT)
        pad = np.zeros((128, H * DH), np.float32)
        Wq = np.concatenate([Wq, pad], 0)
        Wk = np.concatenate([Wk, pad], 0)
        Wv = np.concatenate([Wv, pad], 0)
        Wq[D] = np.asarray(bq, np.float32)
        Wk[D] = np.asarray(bk, np.float32)
        Wv[D] = np.asarray(bv, np.float32)
    consts = _consts_array()
    maskt = mb.T.astype(np.float32) if mode == "general" else None

    in_maps = []
    for c in range(NCORES):
        b, hg = divmod(c, HPC)
        cols = slice(hg * DIMS, (hg + 1) * DIMS)
        xtb = x[b].T
        if has_bias:
            xtb = np.concatenate(
                [xtb, np.ones((1, S), np.float32),
                 np.zeros((127, S), np.float32)], 0)
        im = {
            "xt": _bf16(xtb),
            "wq": _bf16(Wq[:, cols]),
            "wk": _bf16(Wk[:, cols]),
            "wv": _bf16(Wv[:, cols]),
            "wo": _bf16(Wo[hg * DIMS:(hg + 1) * DIMS, :]),
            "cin": _bf16(consts),
        }
        if maskt is not None:
            im["maskt"] = _bf16(maskt)
        in_maps.append(im)
    return in_maps, mode, has_bias


def gather_output(results, bo):
    out = np.zeros((B, S, D), dtype=np.float32)
    for c in range(NCORES):
        out[c // HPC] += np.asarray(results[c]["outp"], dtype=np.float32)
    out += np.asarray(bo, np.float32)[None, None, :]
    return out


def get_program(mode, has_bias, reps=1, phases=None):
    key = (mode, has_bias, reps, phases)
    if key not in _PROG_CACHE:
        _PROG_CACHE[key] = build_program(mode, has_bias, reps, phases)
    return _PROG_CACHE[key]


def run(in_maps, mode, has_bias, reps=1, phases=None, **kwargs):
    from concourse.bass_utils import run_bass_kernel_spmd
    nc = get_program(mode, has_bias, reps, phases)
    return run_bass_kernel_spmd(nc, in_maps, core_ids=list(range(NCORES)),
                                **kwargs)


def kernel(x, mask, Wq, bq, Wk, bk, Wv, bv, Wo, bo):
    in_maps, mode, has_bias = make_in_maps(x, mask, Wq, bq, Wk, bk, Wv, bv,
                                           Wo, bo)
    r = run(in_maps, mode, has_bias, reps=1)
    return gather_output(r.results, bo)
